# revision 19
# baseline (speedup 1.0000x reference)
"""Trainium2 Bass kernel for an explicit-conditioning GRU decoder.

Model (per reference):
  context = encoder_output[-1]                      [B, H]
  step_ids = shift_right(inputs)                    [T, B]
  embs = emb_table[step_ids]                        [T, B, E]
  x_proj = [embs, ctx] @ W_ih.T + b_ih              [T, B, 3H]
  GRU scan over T (h0 = 0)                          hs: [T, B, H]
  logits = [embs, hs, ctx] @ W_out.T + b_out        [T, B, V]
  returns (logits, h_last[None])

Sharding: GRU + x_proj replicated on all 8 cores (the recurrence is the
serial bottleneck; W_hh must stream through each PE every step anyway);
the output projection (V=32000) is sharded over vocab, 4000 rows per
core (padded to 4096).  All device tensors are feature-major (feature
dim on partitions) so every GEMM contracts over partitions with zero
on-device transposes.  Matmuls run in bf16 (host-cast), fp32 PSUM.

Schedule: the GRU's W_hh matmuls are weight-load-bound (N=B=32 moving
columns per 128-column stationary load), leaving the PE stream path
mostly idle.  The x_proj GEMM (next 8-step half-chunk) and the logits
GEMM (vocab tiles over completed 16-step chunks) are interleaved into
each GRU step to fill those gaps.  The ctx-part of the logits GEMM
(identical for every timestep) is computed once in the prologue and
folded in at PSUM evacuation.
"""

from contextlib import ExitStack

import numpy as np
import ml_dtypes

V, E, H = 32000, 512, 1024
T, B, S = 64, 32, 32
NCORES = 8
TB = T * B            # 2048
G3 = 3 * H            # 3072
F = 2 * H + E         # 2560 feature dim of output projection
FEH = E + H           # 1536 emb+hidden feature rows of W_out
VSLICE = V // NCORES  # 4000
VPAD = 4096           # padded vocab slice (32 tiles of 128)
P = 128

BF16 = ml_dtypes.bfloat16

_compiled = {}


def _mkap(base, off_elems, dims):
    """Raw access pattern on base AP's tensor: dims = [[stride, count], ...]."""
    import concourse.bass as bass

    return bass.AP(tensor=base.tensor, offset=base.offset + off_elems, ap=dims)


def _bcast(tile3, m, reps):
    """[128, M, B] tile -> [:, m, :] broadcast to [128, reps, B] (stride 0)."""
    import concourse.bass as bass

    s = tile3[:, m, :]
    return bass.AP(tensor=s.tensor, offset=s.offset,
                   ap=[s.ap[0], [0, reps], s.ap[1]])


def _build():
    import concourse.bacc as bacc
    import concourse.bass as bass
    import concourse.mybir as mybir
    import concourse.tile as tile

    fp32 = mybir.dt.float32
    bf16 = mybir.dt.bfloat16
    AF = mybir.ActivationFunctionType
    OP = mybir.AluOpType
    ts = bass.ts

    nc = bacc.Bacc("TRN2", target_bir_lowering=False, debug=False)

    # ---- I/O ----
    embsT = nc.dram_tensor("embsT", [E, TB], bf16, kind="ExternalInput")
    ctxT = nc.dram_tensor("ctxT", [H, B], bf16, kind="ExternalInput")
    wihT_emb = nc.dram_tensor("wihT_emb", [E, G3], bf16, kind="ExternalInput")
    wihT_ctx = nc.dram_tensor("wihT_ctx", [H, G3], bf16, kind="ExternalInput")
    whhT = nc.dram_tensor("whhT", [H, G3], bf16, kind="ExternalInput")
    woutT = nc.dram_tensor("woutT", [F, VPAD], bf16, kind="ExternalInput")
    cb_bias = nc.dram_tensor("cb_bias", [G3], fp32, kind="ExternalInput")
    bhhn = nc.dram_tensor("bhhn", [H, B], fp32, kind="ExternalInput")
    bout = nc.dram_tensor("bout", [VPAD], fp32, kind="ExternalInput")

    logits_out = nc.dram_tensor("logits_out", [VPAD, TB], fp32, kind="ExternalOutput")
    h_out = nc.dram_tensor("h_out", [8, P, B], fp32, kind="ExternalOutput")

    KC_E = E // P      # 4
    KC_H = H // P      # 8
    MC_G = G3 // P     # 24
    MC_V = VPAD // P   # 32
    KC_L = FEH // P    # 12 emb+hs K-chunks of the logits GEMM
    NHC = T // 8       # 8 half-chunks of 8 steps (256 tb cols)

    with tile.TileContext(nc) as tc:
        with (
            tc.tile_pool(name="const", bufs=1) as const,
            tc.tile_pool(name="xp", bufs=2) as xpp,
            tc.tile_pool(name="gate", bufs=2) as gate,
            tc.tile_pool(name="wv", bufs=2) as wvp,
            tc.tile_pool(name="lv", bufs=2) as lvp,
            tc.tile_pool(name="ps_g", bufs=2, space="PSUM") as ps_g,
            tc.tile_pool(name="ps_b", bufs=2, space="PSUM") as ps_b,
            tc.tile_pool(name="ps_x", bufs=2, space="PSUM") as ps_x,
        ):
            # ================= constants into SBUF =================
            whh_sb = const.tile([P, KC_H, G3], bf16)
            nc.scalar.dma_start(
                out=whh_sb, in_=whhT[:].rearrange("(k p) g -> p k g", p=P)
            )
            embs_sb = const.tile([P, KC_E, TB], bf16)
            nc.gpsimd.dma_start(
                out=embs_sb, in_=embsT[:].rearrange("(k p) n -> p k n", p=P)
            )
            wemb_sb = const.tile([P, KC_E, G3], bf16)
            nc.gpsimd.dma_start(
                out=wemb_sb, in_=wihT_emb[:].rearrange("(k p) g -> p k g", p=P)
            )
            ctxk_sb = const.tile([P, KC_H, B], bf16)
            nc.sync.dma_start(
                out=ctxk_sb, in_=ctxT[:].rearrange("(k p) b -> p k b", p=P)
            )
            bmat_sb = const.tile([P, KC_H, B], fp32)
            nc.sync.dma_start(
                out=bmat_sb, in_=bhhn[:].rearrange("(k p) b -> p k b", p=P)
            )
            bias24_sb = const.tile([P, MC_G], fp32)
            nc.sync.dma_start(
                out=bias24_sb, in_=_mkap(cb_bias[:], 0, [[1, P], [P, MC_G]])
            )
            bout_sb = const.tile([P, MC_V], fp32)
            nc.sync.dma_start(
                out=bout_sb, in_=_mkap(bout[:], 0, [[1, P], [P, MC_V]])
            )

            hsT = const.tile([P, KC_H, TB], bf16)
            cb24_sb = const.tile([P, MC_G, B], bf16)    # ctx_proj + biases
            outctx_sb = const.tile([P, MC_V, B], bf16)  # logits ctx part

            # ============ prologue: ctx projection + step bias ============
            for grp in range(3):
                with tc.tile_pool(name="wtmp", bufs=1) as wtmp:
                    wctx_sb = wtmp.tile([P, KC_H, 1024], bf16, name="wctx_sb")
                    nc.sync.dma_start(
                        out=wctx_sb,
                        in_=_mkap(
                            wihT_ctx[:], grp * 1024,
                            [[G3, P], [P * G3, KC_H], [1, 1024]],
                        ),
                    )
                    for mm in range(8):
                        m = grp * 8 + mm
                        ps = ps_x.tile([P, 256], fp32, tag="psx")
                        for k in range(KC_H):
                            nc.tensor.matmul(
                                ps[:, :B],
                                wctx_sb[:, k, ts(mm, P)],
                                ctxk_sb[:, k, :],
                                start=(k == 0),
                                stop=(k == KC_H - 1),
                            )
                        nc.vector.tensor_scalar_add(
                            out=cb24_sb[:, m, :], in0=ps[:, :B],
                            scalar1=bias24_sb[:, m : m + 1],
                        )

            # ============ prologue: logits ctx part (t-invariant) ============
            for grp in range(4):
                with tc.tile_pool(name="wco", bufs=1) as wcop:
                    wco_sb = wcop.tile([P, KC_H, 1024], bf16, name="wco_sb")
                    nc.sync.dma_start(
                        out=wco_sb,
                        in_=_mkap(
                            woutT[:], FEH * VPAD + grp * 1024,
                            [[VPAD, P], [P * VPAD, KC_H], [1, 1024]],
                        ),
                    )
                    for mm in range(8):
                        m = grp * 8 + mm
                        ps = ps_x.tile([P, 256], fp32, tag="psx")
                        for k in range(KC_H):
                            nc.tensor.matmul(
                                ps[:, :B],
                                wco_sb[:, k, ts(mm, P)],
                                ctxk_sb[:, k, :],
                                start=(k == 0),
                                stop=(k == KC_H - 1),
                            )
                        nc.vector.tensor_copy(
                            out=outctx_sb[:, m, :], in_=ps[:, :B]
                        )

            # ============ x_proj half-chunk emitter ============
            xp_tiles = {}

            def emit_xproj(hc, m):
                if m == 0:
                    xp_tiles[hc] = xpp.tile([P, KC_H, 8, 96], bf16, tag="xp", name=f"xp{hc}")
                xp = xp_tiles[hc]
                g, j = divmod(m, KC_H)
                ps = ps_x.tile([P, 256], fp32, tag="psx")
                for k in range(KC_E):
                    nc.tensor.matmul(
                        ps,
                        wemb_sb[:, k, ts(m, P)],
                        embs_sb[:, k, hc * 256 : (hc + 1) * 256],
                        start=(k == 0),
                        stop=(k == KC_E - 1),
                    )
                nc.vector.tensor_tensor(
                    out=xp[:, j, :, ts(g, B)],
                    in0=ps.rearrange("p (t b) -> p t b", b=B),
                    in1=_bcast(cb24_sb, m, 8),
                    op=OP.add,
                )

            for m in range(MC_G):
                emit_xproj(0, m)

            # ============ logits half-pair emitter (N=256) ============
            def emit_logits_pair(q):
                hn, m = divmod(q, MC_V)
                wv = wvp.tile([P, KC_L, P], bf16, tag="wv", name=f"wv{q}")
                nc.sync.dma_start(
                    out=wv,
                    in_=_mkap(
                        woutT[:], m * P, [[VPAD, P], [P * VPAD, KC_L], [1, P]]
                    ),
                )
                ps = ps_b.tile([P, 256], fp32, tag="psb", name=f"lps{q}")
                for k in range(KC_L):
                    if k < KC_E:
                        rhs = embs_sb[:, k, hn * 256 : (hn + 1) * 256]
                    else:
                        rhs = hsT[:, k - KC_E, hn * 256 : (hn + 1) * 256]
                    nc.tensor.matmul(
                        ps, wv[:, k, :], rhs,
                        start=(k == 0), stop=(k == KC_L - 1),
                    )
                lv = lvp.tile([P, 8, B], fp32, tag="lv", name=f"lv{q}")
                nc.vector.scalar_tensor_tensor(
                    out=lv,
                    in0=ps.rearrange("p (t b) -> p t b", b=B),
                    scalar=bout_sb[:, m : m + 1],
                    in1=_bcast(outctx_sb, m, 8),
                    op0=OP.add, op1=OP.add,
                )
                nc.sync.dma_start(
                    out=_mkap(
                        logits_out[:], m * P * TB + hn * 256, [[TB, P], [1, 256]]
                    ),
                    in_=lv,
                )

            # ================= GRU with interleaved fillers =================
            for t in range(T):
                hc, tcc = divmod(t, 8)
                xp = xp_tiles[hc]
                if t > 0:
                    psg = ps_g.tile([P, KC_H, P], fp32, tag="psg")
                    for j in range(KC_H):
                        for g in range(3):
                            for k in range(KC_H):
                                nc.tensor.matmul(
                                    psg[:, j, ts(g, B)],
                                    whh_sb[:, k, g * H + j * P : g * H + (j + 1) * P],
                                    hsT[:, k, ts(t - 1, B)],
                                    start=(j % 4 == 0 and g == 0 and k == 0),
                                    stop=(j % 4 == 3 and g == 2 and k == KC_H - 1),
                                )
                    prz = gate.tile([P, KC_H, 64], fp32, tag="prz")
                    nc.vector.tensor_tensor(
                        out=prz, in0=psg[:, :, 0:64], in1=xp[:, :, tcc, 0:64],
                        op=OP.add,
                    )
                    rz = gate.tile([P, KC_H, 64], bf16, tag="rz")
                    nc.scalar.activation(out=rz, in_=prz, func=AF.Sigmoid)
                    omz = gate.tile([P, KC_H, B], bf16, tag="omz")
                    nc.vector.tensor_scalar(
                        out=omz, in0=rz[:, :, B:64], scalar1=-1.0, scalar2=1.0,
                        op0=OP.mult, op1=OP.add,
                    )
                    w = gate.tile([P, KC_H, B], bf16, tag="w")
                    nc.vector.tensor_tensor(
                        out=w, in0=rz[:, :, B:64], in1=hsT[:, :, ts(t - 1, B)],
                        op=OP.mult,
                    )
                    s1 = gate.tile([P, KC_H, B], fp32, tag="s1")
                    nc.vector.tensor_tensor(
                        out=s1, in0=psg[:, :, 64:96], in1=bmat_sb, op=OP.add
                    )
                    nc.vector.tensor_tensor(
                        out=s1, in0=s1, in1=rz[:, :, 0:B], op=OP.mult
                    )
                    nc.vector.tensor_tensor(
                        out=s1, in0=s1, in1=xp[:, :, tcc, 64:96], op=OP.add
                    )
                    n_ = gate.tile([P, KC_H, B], bf16, tag="n_")
                    nc.scalar.activation(out=n_, in_=s1, func=AF.Tanh)
                    u = gate.tile([P, KC_H, B], bf16, tag="u")
                    nc.vector.tensor_tensor(out=u, in0=n_, in1=omz, op=OP.mult)
                    nc.vector.tensor_tensor(
                        out=hsT[:, :, ts(t, B)], in0=u, in1=w, op=OP.add
                    )
                    if t == T - 1:
                        hf = gate.tile([P, KC_H, B], fp32, tag="hf")
                        nc.vector.tensor_tensor(out=hf, in0=u, in1=w, op=OP.add)
                        nc.sync.dma_start(
                            out=_mkap(h_out[:], 0, [[B, P], [P * B, KC_H], [1, B]]),
                            in_=hf,
                        )
                else:
                    # h0 = 0: gates from x_proj only; hn = b_hh_n
                    rz = gate.tile([P, KC_H, 64], bf16, tag="rz")
                    nc.scalar.activation(
                        out=rz, in_=xp[:, :, 0, 0:64], func=AF.Sigmoid
                    )
                    omz = gate.tile([P, KC_H, B], bf16, tag="omz")
                    nc.vector.tensor_scalar(
                        out=omz, in0=rz[:, :, B:64], scalar1=-1.0, scalar2=1.0,
                        op0=OP.mult, op1=OP.add,
                    )
                    s1 = gate.tile([P, KC_H, B], fp32, tag="s1")
                    nc.vector.tensor_tensor(
                        out=s1, in0=bmat_sb, in1=rz[:, :, 0:B], op=OP.mult
                    )
                    nc.vector.tensor_tensor(
                        out=s1, in0=s1, in1=xp[:, :, 0, 64:96], op=OP.add
                    )
                    n_ = gate.tile([P, KC_H, B], bf16, tag="n_")
                    nc.scalar.activation(out=n_, in_=s1, func=AF.Tanh)
                    nc.vector.tensor_tensor(
                        out=hsT[:, :, ts(t, B)], in0=n_, in1=omz, op=OP.mult
                    )

                # -------- fillers: x_proj for next half-chunk, logits --------
                if hc + 1 < NHC:
                    for m in range(3 * tcc, 3 * (tcc + 1)):
                        emit_xproj(hc + 1, m)
                if t >= 8:
                    for i in range(4):
                        emit_logits_pair(4 * (t - 8) + i)

            # tail: last half-chunk (needs steps 56-63 complete)
            for q in range(224, 256):
                emit_logits_pair(q)

    nc.compile()
    return nc


def _get_nc():
    if "nc" not in _compiled:
        _compiled["nc"] = _build()
    return _compiled["nc"]


def _enable_tracing():
    """Dev-only: install the axon NTFF profile hook (the container's antenv
    stub lacks axon_hooks) and stub out the artifact upload."""
    import sys
    import types

    from concourse import bass_utils

    bass_utils.upload_artifacts = lambda tmpdir: "local://" + tmpdir
    try:
        from antenv.axon_hooks import get_axon_ntff_profile_hook  # noqa: F401
        return
    except ImportError:
        pass
    mod = types.ModuleType("antenv.axon_hooks")
    _hook = [None]
    mod.set_axon_ntff_profile_hook = lambda h: _hook.__setitem__(0, h)
    mod.get_axon_ntff_profile_hook = lambda: _hook[0]
    sys.modules["antenv.axon_hooks"] = mod
    import antenv

    antenv.axon_hooks = mod
    from trn_agent_boot import trn_boot

    mod.set_axon_ntff_profile_hook(
        trn_boot._ntff_profile_via_ctypes("/opt/axon/libaxon_pjrt.so")
    )


def kernel(inputs, encoder_output, encoder_mask, emb_table,
           W_ih, W_hh, b_ih, b_hh, W_out, b_out):
    import os
    from concourse import bass_utils

    inputs = np.asarray(inputs)
    encoder_output = np.asarray(encoder_output, dtype=np.float32)
    emb_table = np.asarray(emb_table, dtype=np.float32)
    W_ih = np.asarray(W_ih, dtype=np.float32)
    W_hh = np.asarray(W_hh, dtype=np.float32)
    b_ih = np.asarray(b_ih, dtype=np.float32)
    b_hh = np.asarray(b_hh, dtype=np.float32)
    W_out = np.asarray(W_out, dtype=np.float32)
    b_out = np.asarray(b_out, dtype=np.float32)

    # host-side prep: shift ids, gather embeddings, feature-major + bf16
    step_ids = np.concatenate([inputs[:1], inputs[:-1]], axis=0).reshape(-1)
    embs = emb_table[step_ids]                                   # [TB, E]
    embsT = np.ascontiguousarray(embs.T).astype(BF16)            # [E, TB]
    ctxT = np.ascontiguousarray(encoder_output[-1].T).astype(BF16)  # [H, B]
    wihT = np.ascontiguousarray(W_ih.T)                          # [E+H, 3H]
    wihT_emb = wihT[:E].astype(BF16)
    wihT_ctx = wihT[E:].astype(BF16)
    whhT = np.ascontiguousarray(W_hh.T).astype(BF16)             # [H, 3H]
    woutT = np.ascontiguousarray(W_out.T)                        # [F, V]
    cb = b_ih.copy()
    cb[: 2 * H] += b_hh[: 2 * H]
    bhhn = np.ascontiguousarray(
        np.broadcast_to(b_hh[2 * H :, None], (H, B))
    )

    common = dict(
        embsT=embsT, ctxT=ctxT, wihT_emb=wihT_emb, wihT_ctx=wihT_ctx,
        whhT=whhT, cb_bias=cb, bhhn=bhhn,
    )
    in_maps = []
    for i in range(NCORES):
        wslice = np.zeros((F, VPAD), dtype=BF16)
        wslice[:, :VSLICE] = woutT[:, i * VSLICE : (i + 1) * VSLICE].astype(BF16)
        bslice = np.zeros((VPAD,), dtype=np.float32)
        bslice[:VSLICE] = b_out[i * VSLICE : (i + 1) * VSLICE]
        in_maps.append(dict(common, woutT=wslice, bout=bslice))

    nc = _get_nc()
    trace = bool(int(os.environ.get("KERNEL_TRACE", "0")))
    kw = {}
    if trace:
        _enable_tracing()
        tmpdir = os.environ.get("KERNEL_TRACE_DIR")
        if tmpdir:
            os.makedirs(tmpdir, exist_ok=True)
            kw["tmpdir"] = tmpdir
    res = bass_utils.run_bass_kernel_spmd(
        nc, in_maps, core_ids=list(range(NCORES)), trace=trace, **kw
    )
    _compiled["last_exec_ns"] = res.exec_time_ns
    _compiled["last_result"] = res

    logits = np.empty((T, B, V), dtype=np.float32)
    for i in range(NCORES):
        li = res.results[i]["logits_out"][:VSLICE]               # [4000, TB]
        logits[:, :, i * VSLICE : (i + 1) * VSLICE] = (
            li.reshape(VSLICE, T, B).transpose(1, 2, 0)
        )
    hidden = res.results[0]["h_out"].reshape(H, B).T[None]       # [1, B, H]
    return logits, hidden


# revision 20
# speedup vs baseline: 1.1350x; 1.1350x over previous
"""Trainium2 Bass kernel for an explicit-conditioning GRU decoder.

Model (per reference):
  context = encoder_output[-1]                      [B, H]
  step_ids = shift_right(inputs)                    [T, B]
  embs = emb_table[step_ids]                        [T, B, E]
  x_proj = [embs, ctx] @ W_ih.T + b_ih              [T, B, 3H]
  GRU scan over T (h0 = 0)                          hs: [T, B, H]
  logits = [embs, hs, ctx] @ W_out.T + b_out        [T, B, V]
  returns (logits, h_last[None])

Sharding: GRU + x_proj replicated on all 8 cores (the recurrence is the
serial bottleneck; W_hh must stream through each PE every step anyway);
the output projection (V=32000) is sharded over vocab, 4000 rows per
core (padded to 4096).  All device tensors are feature-major (feature
dim on partitions) so every GEMM contracts over partitions with zero
on-device transposes.  Matmuls run in bf16 (host-cast), fp32 PSUM.

Schedule: the GRU's W_hh matmuls are weight-load-bound (N=B=32 moving
columns per 128-column stationary load), leaving the PE stream path
mostly idle.  The x_proj GEMM (next 8-step half-chunk) and the logits
GEMM (vocab tiles over completed 16-step chunks) are interleaved into
each GRU step to fill those gaps.  The ctx-part of the logits GEMM
(identical for every timestep) is computed once in the prologue and
folded in at PSUM evacuation.
"""

from contextlib import ExitStack

import numpy as np
import ml_dtypes

V, E, H = 32000, 512, 1024
T, B, S = 64, 32, 32
NCORES = 8
TB = T * B            # 2048
G3 = 3 * H            # 3072
F = 2 * H + E         # 2560 feature dim of output projection
FEH = E + H           # 1536 emb+hidden feature rows of W_out
VSLICE = V // NCORES  # 4000
VPAD = 4096           # padded vocab slice (32 tiles of 128)
P = 128

BF16 = ml_dtypes.bfloat16

_compiled = {}


def _mkap(base, off_elems, dims):
    """Raw access pattern on base AP's tensor: dims = [[stride, count], ...]."""
    import concourse.bass as bass

    return bass.AP(tensor=base.tensor, offset=base.offset + off_elems, ap=dims)


def _bcast(tile3, m, reps):
    """[128, M, B] tile -> [:, m, :] broadcast to [128, reps, B] (stride 0)."""
    import concourse.bass as bass

    s = tile3[:, m, :]
    return bass.AP(tensor=s.tensor, offset=s.offset,
                   ap=[s.ap[0], [0, reps], s.ap[1]])


def _build():
    import concourse.bacc as bacc
    import concourse.bass as bass
    import concourse.mybir as mybir
    import concourse.tile as tile

    fp32 = mybir.dt.float32
    bf16 = mybir.dt.bfloat16
    AF = mybir.ActivationFunctionType
    OP = mybir.AluOpType
    ts = bass.ts

    nc = bacc.Bacc("TRN2", target_bir_lowering=False, debug=False)

    # ---- I/O ----
    embsT = nc.dram_tensor("embsT", [E, TB], bf16, kind="ExternalInput")
    ctxT = nc.dram_tensor("ctxT", [H, B], bf16, kind="ExternalInput")
    wihT_emb = nc.dram_tensor("wihT_emb", [E, G3], bf16, kind="ExternalInput")
    wihT_ctx = nc.dram_tensor("wihT_ctx", [H, G3], bf16, kind="ExternalInput")
    whhT = nc.dram_tensor("whhT", [H, G3], bf16, kind="ExternalInput")
    woutT = nc.dram_tensor("woutT", [F, VPAD], bf16, kind="ExternalInput")
    cb_bias = nc.dram_tensor("cb_bias", [G3], fp32, kind="ExternalInput")
    bhhn = nc.dram_tensor("bhhn", [H, B], fp32, kind="ExternalInput")
    bout = nc.dram_tensor("bout", [VPAD], fp32, kind="ExternalInput")

    logits_out = nc.dram_tensor("logits_out", [VPAD, TB], fp32, kind="ExternalOutput")
    h_out = nc.dram_tensor("h_out", [8, P, B], fp32, kind="ExternalOutput")

    KC_E = E // P      # 4
    KC_H = H // P      # 8
    MC_G = G3 // P     # 24
    MC_V = VPAD // P   # 32
    KC_L = FEH // P    # 12 emb+hs K-chunks of the logits GEMM
    NHC = T // 8       # 8 half-chunks of 8 steps (256 tb cols)

    with tile.TileContext(nc) as tc:
        with (
            tc.tile_pool(name="const", bufs=1) as const,
            tc.tile_pool(name="xp", bufs=2) as xpp,
            tc.tile_pool(name="gate", bufs=2) as gate,
            tc.tile_pool(name="wv", bufs=2) as wvp,
            tc.tile_pool(name="lv", bufs=2) as lvp,
            tc.tile_pool(name="ps_g", bufs=2, space="PSUM") as ps_g,
            tc.tile_pool(name="ps_b", bufs=2, space="PSUM") as ps_b,
            tc.tile_pool(name="ps_x", bufs=2, space="PSUM") as ps_x,
        ):
            # ================= constants into SBUF =================
            whh_sb = const.tile([P, KC_H, G3], bf16)
            nc.sync.dma_start(
                out=whh_sb, in_=whhT[:].rearrange("(k p) g -> p k g", p=P)
            )
            embs_sb = const.tile([P, KC_E, TB], bf16)
            nc.sync.dma_start(
                out=embs_sb, in_=embsT[:].rearrange("(k p) n -> p k n", p=P)
            )
            wemb_sb = const.tile([P, KC_E, G3], bf16)
            nc.sync.dma_start(
                out=wemb_sb, in_=wihT_emb[:].rearrange("(k p) g -> p k g", p=P)
            )
            ctxk_sb = const.tile([P, KC_H, B], bf16)
            nc.sync.dma_start(
                out=ctxk_sb, in_=ctxT[:].rearrange("(k p) b -> p k b", p=P)
            )
            bmat_sb = const.tile([P, KC_H, B], fp32)
            nc.sync.dma_start(
                out=bmat_sb, in_=bhhn[:].rearrange("(k p) b -> p k b", p=P)
            )
            bias24_sb = const.tile([P, MC_G], fp32)
            nc.sync.dma_start(
                out=bias24_sb, in_=_mkap(cb_bias[:], 0, [[1, P], [P, MC_G]])
            )
            bout_sb = const.tile([P, MC_V], fp32)
            nc.sync.dma_start(
                out=bout_sb, in_=_mkap(bout[:], 0, [[1, P], [P, MC_V]])
            )

            hsT = const.tile([P, KC_H, TB], bf16)
            cb24_sb = const.tile([P, MC_G, B], bf16)    # ctx_proj + biases
            outctx_sb = const.tile([P, MC_V, B], bf16)  # logits ctx part

            # ============ prologue: ctx projection + step bias ============
            for grp in range(3):
                with tc.tile_pool(name="wtmp", bufs=1) as wtmp:
                    wctx_sb = wtmp.tile([P, KC_H, 1024], bf16, name="wctx_sb")
                    nc.sync.dma_start(
                        out=wctx_sb,
                        in_=_mkap(
                            wihT_ctx[:], grp * 1024,
                            [[G3, P], [P * G3, KC_H], [1, 1024]],
                        ),
                    )
                    for mm in range(8):
                        m = grp * 8 + mm
                        ps = ps_x.tile([P, 256], fp32, tag="psx")
                        for k in range(KC_H):
                            nc.tensor.matmul(
                                ps[:, :B],
                                wctx_sb[:, k, ts(mm, P)],
                                ctxk_sb[:, k, :],
                                start=(k == 0),
                                stop=(k == KC_H - 1),
                            )
                        nc.vector.tensor_scalar_add(
                            out=cb24_sb[:, m, :], in0=ps[:, :B],
                            scalar1=bias24_sb[:, m : m + 1],
                        )

            # ============ prologue: logits ctx part (t-invariant) ============
            for grp in range(4):
                with tc.tile_pool(name="wco", bufs=1) as wcop:
                    wco_sb = wcop.tile([P, KC_H, 1024], bf16, name="wco_sb")
                    nc.sync.dma_start(
                        out=wco_sb,
                        in_=_mkap(
                            woutT[:], FEH * VPAD + grp * 1024,
                            [[VPAD, P], [P * VPAD, KC_H], [1, 1024]],
                        ),
                    )
                    for mm in range(8):
                        m = grp * 8 + mm
                        ps = ps_x.tile([P, 256], fp32, tag="psx")
                        for k in range(KC_H):
                            nc.tensor.matmul(
                                ps[:, :B],
                                wco_sb[:, k, ts(mm, P)],
                                ctxk_sb[:, k, :],
                                start=(k == 0),
                                stop=(k == KC_H - 1),
                            )
                        nc.vector.tensor_copy(
                            out=outctx_sb[:, m, :], in_=ps[:, :B]
                        )

            # ============ x_proj half-chunk emitter ============
            xp_tiles = {}

            def emit_xproj(hc, m):
                if m == 0:
                    xp_tiles[hc] = xpp.tile([P, KC_H, 8, 96], bf16, tag="xp", name=f"xp{hc}")
                xp = xp_tiles[hc]
                g, j = divmod(m, KC_H)
                ps = ps_x.tile([P, 256], fp32, tag="psx")
                for k in range(KC_E):
                    nc.tensor.matmul(
                        ps,
                        wemb_sb[:, k, ts(m, P)],
                        embs_sb[:, k, hc * 256 : (hc + 1) * 256],
                        start=(k == 0),
                        stop=(k == KC_E - 1),
                    )
                nc.vector.tensor_tensor(
                    out=xp[:, j, :, ts(g, B)],
                    in0=ps.rearrange("p (t b) -> p t b", b=B),
                    in1=_bcast(cb24_sb, m, 8),
                    op=OP.add,
                )

            for m in range(MC_G):
                emit_xproj(0, m)

            # ============ logits pair emitter ============
            def emit_logits_pair(q):
                n, m = divmod(q, MC_V)
                wv = wvp.tile([P, KC_L, P], bf16, tag="wv")
                nc.sync.dma_start(
                    out=wv,
                    in_=_mkap(
                        woutT[:], m * P, [[VPAD, P], [P * VPAD, KC_L], [1, P]]
                    ),
                )
                ps = ps_b.tile([P, 512], fp32, tag="psb")
                for k in range(KC_L):
                    if k < KC_E:
                        rhs = embs_sb[:, k, ts(n, 512)]
                    else:
                        rhs = hsT[:, k - KC_E, ts(n, 512)]
                    nc.tensor.matmul(
                        ps, wv[:, k, :], rhs,
                        start=(k == 0), stop=(k == KC_L - 1),
                    )
                lv = lvp.tile([P, 16, B], fp32, tag="lv")
                nc.vector.scalar_tensor_tensor(
                    out=lv,
                    in0=ps.rearrange("p (t b) -> p t b", b=B),
                    scalar=bout_sb[:, m : m + 1],
                    in1=_bcast(outctx_sb, m, 16),
                    op0=OP.add, op1=OP.add,
                )
                nc.sync.dma_start(
                    out=_mkap(
                        logits_out[:], m * P * TB + n * 512, [[TB, P], [1, 512]]
                    ),
                    in_=lv,
                )

            # ================= GRU with interleaved fillers =================
            for t in range(T):
                hc, tcc = divmod(t, 8)
                xp = xp_tiles[hc]
                if t > 0:
                    psg = ps_g.tile([P, KC_H, P], fp32, tag="psg")
                    for j in range(KC_H):
                        for g in range(3):
                            for k in range(KC_H):
                                nc.tensor.matmul(
                                    psg[:, j, ts(g, B)],
                                    whh_sb[:, k, g * H + j * P : g * H + (j + 1) * P],
                                    hsT[:, k, ts(t - 1, B)],
                                    start=(j % 4 == 0 and g == 0 and k == 0),
                                    stop=(j % 4 == 3 and g == 2 and k == KC_H - 1),
                                )
                    prz = gate.tile([P, KC_H, 64], fp32, tag="prz")
                    nc.vector.tensor_tensor(
                        out=prz, in0=psg[:, :, 0:64], in1=xp[:, :, tcc, 0:64],
                        op=OP.add,
                    )
                    rz = gate.tile([P, KC_H, 64], bf16, tag="rz")
                    nc.scalar.activation(out=rz, in_=prz, func=AF.Sigmoid)
                    omz = gate.tile([P, KC_H, B], bf16, tag="omz")
                    nc.vector.tensor_scalar(
                        out=omz, in0=rz[:, :, B:64], scalar1=-1.0, scalar2=1.0,
                        op0=OP.mult, op1=OP.add,
                    )
                    w = gate.tile([P, KC_H, B], bf16, tag="w")
                    nc.vector.tensor_tensor(
                        out=w, in0=rz[:, :, B:64], in1=hsT[:, :, ts(t - 1, B)],
                        op=OP.mult,
                    )
                    s1 = gate.tile([P, KC_H, B], fp32, tag="s1")
                    nc.vector.tensor_tensor(
                        out=s1, in0=psg[:, :, 64:96], in1=bmat_sb, op=OP.add
                    )
                    nc.vector.tensor_tensor(
                        out=s1, in0=s1, in1=rz[:, :, 0:B], op=OP.mult
                    )
                    nc.vector.tensor_tensor(
                        out=s1, in0=s1, in1=xp[:, :, tcc, 64:96], op=OP.add
                    )
                    n_ = gate.tile([P, KC_H, B], bf16, tag="n_")
                    nc.scalar.activation(out=n_, in_=s1, func=AF.Tanh)
                    u = gate.tile([P, KC_H, B], bf16, tag="u")
                    nc.vector.tensor_tensor(out=u, in0=n_, in1=omz, op=OP.mult)
                    nc.vector.tensor_tensor(
                        out=hsT[:, :, ts(t, B)], in0=u, in1=w, op=OP.add
                    )
                    if t == T - 1:
                        hf = gate.tile([P, KC_H, B], fp32, tag="hf")
                        nc.vector.tensor_tensor(out=hf, in0=u, in1=w, op=OP.add)
                        nc.sync.dma_start(
                            out=_mkap(h_out[:], 0, [[B, P], [P * B, KC_H], [1, B]]),
                            in_=hf,
                        )
                else:
                    # h0 = 0: gates from x_proj only; hn = b_hh_n
                    rz = gate.tile([P, KC_H, 64], bf16, tag="rz")
                    nc.scalar.activation(
                        out=rz, in_=xp[:, :, 0, 0:64], func=AF.Sigmoid
                    )
                    omz = gate.tile([P, KC_H, B], bf16, tag="omz")
                    nc.vector.tensor_scalar(
                        out=omz, in0=rz[:, :, B:64], scalar1=-1.0, scalar2=1.0,
                        op0=OP.mult, op1=OP.add,
                    )
                    s1 = gate.tile([P, KC_H, B], fp32, tag="s1")
                    nc.vector.tensor_tensor(
                        out=s1, in0=bmat_sb, in1=rz[:, :, 0:B], op=OP.mult
                    )
                    nc.vector.tensor_tensor(
                        out=s1, in0=s1, in1=xp[:, :, 0, 64:96], op=OP.add
                    )
                    n_ = gate.tile([P, KC_H, B], bf16, tag="n_")
                    nc.scalar.activation(out=n_, in_=s1, func=AF.Tanh)
                    nc.vector.tensor_tensor(
                        out=hsT[:, :, ts(t, B)], in0=n_, in1=omz, op=OP.mult
                    )

                # -------- fillers: x_proj for next half-chunk, logits --------
                if hc + 1 < NHC:
                    for m in range(3 * tcc, 3 * (tcc + 1)):
                        emit_xproj(hc + 1, m)
                if t >= 16:
                    emit_logits_pair(2 * (t - 16))
                    emit_logits_pair(2 * (t - 16) + 1)

            # tail: last vocab chunk (needs steps 48-63 complete)
            for q in range(96, 128):
                emit_logits_pair(q)

    nc.compile()
    return nc


def _get_nc():
    if "nc" not in _compiled:
        _compiled["nc"] = _build()
    return _compiled["nc"]


def _enable_tracing():
    """Dev-only: install the axon NTFF profile hook (the container's antenv
    stub lacks axon_hooks) and stub out the artifact upload."""
    import sys
    import types

    from concourse import bass_utils

    bass_utils.upload_artifacts = lambda tmpdir: "local://" + tmpdir
    try:
        from antenv.axon_hooks import get_axon_ntff_profile_hook  # noqa: F401
        return
    except ImportError:
        pass
    mod = types.ModuleType("antenv.axon_hooks")
    _hook = [None]
    mod.set_axon_ntff_profile_hook = lambda h: _hook.__setitem__(0, h)
    mod.get_axon_ntff_profile_hook = lambda: _hook[0]
    sys.modules["antenv.axon_hooks"] = mod
    import antenv

    antenv.axon_hooks = mod
    from trn_agent_boot import trn_boot

    mod.set_axon_ntff_profile_hook(
        trn_boot._ntff_profile_via_ctypes("/opt/axon/libaxon_pjrt.so")
    )


def kernel(inputs, encoder_output, encoder_mask, emb_table,
           W_ih, W_hh, b_ih, b_hh, W_out, b_out):
    import os
    from concourse import bass_utils

    inputs = np.asarray(inputs)
    encoder_output = np.asarray(encoder_output, dtype=np.float32)
    emb_table = np.asarray(emb_table, dtype=np.float32)
    W_ih = np.asarray(W_ih, dtype=np.float32)
    W_hh = np.asarray(W_hh, dtype=np.float32)
    b_ih = np.asarray(b_ih, dtype=np.float32)
    b_hh = np.asarray(b_hh, dtype=np.float32)
    W_out = np.asarray(W_out, dtype=np.float32)
    b_out = np.asarray(b_out, dtype=np.float32)

    # host-side prep: shift ids, gather embeddings, feature-major + bf16
    step_ids = np.concatenate([inputs[:1], inputs[:-1]], axis=0).reshape(-1)
    embs = emb_table[step_ids]                                   # [TB, E]
    embsT = np.ascontiguousarray(embs.T).astype(BF16)            # [E, TB]
    ctxT = np.ascontiguousarray(encoder_output[-1].T).astype(BF16)  # [H, B]
    wihT = np.ascontiguousarray(W_ih.T)                          # [E+H, 3H]
    wihT_emb = wihT[:E].astype(BF16)
    wihT_ctx = wihT[E:].astype(BF16)
    whhT = np.ascontiguousarray(W_hh.T).astype(BF16)             # [H, 3H]
    woutT = np.ascontiguousarray(W_out.T)                        # [F, V]
    cb = b_ih.copy()
    cb[: 2 * H] += b_hh[: 2 * H]
    bhhn = np.ascontiguousarray(
        np.broadcast_to(b_hh[2 * H :, None], (H, B))
    )

    common = dict(
        embsT=embsT, ctxT=ctxT, wihT_emb=wihT_emb, wihT_ctx=wihT_ctx,
        whhT=whhT, cb_bias=cb, bhhn=bhhn,
    )
    in_maps = []
    for i in range(NCORES):
        wslice = np.zeros((F, VPAD), dtype=BF16)
        wslice[:, :VSLICE] = woutT[:, i * VSLICE : (i + 1) * VSLICE].astype(BF16)
        bslice = np.zeros((VPAD,), dtype=np.float32)
        bslice[:VSLICE] = b_out[i * VSLICE : (i + 1) * VSLICE]
        in_maps.append(dict(common, woutT=wslice, bout=bslice))

    nc = _get_nc()
    trace = bool(int(os.environ.get("KERNEL_TRACE", "0")))
    kw = {}
    if trace:
        _enable_tracing()
        tmpdir = os.environ.get("KERNEL_TRACE_DIR")
        if tmpdir:
            os.makedirs(tmpdir, exist_ok=True)
            kw["tmpdir"] = tmpdir
    res = bass_utils.run_bass_kernel_spmd(
        nc, in_maps, core_ids=list(range(NCORES)), trace=trace, **kw
    )
    _compiled["last_exec_ns"] = res.exec_time_ns
    _compiled["last_result"] = res

    logits = np.empty((T, B, V), dtype=np.float32)
    for i in range(NCORES):
        li = res.results[i]["logits_out"][:VSLICE]               # [4000, TB]
        logits[:, :, i * VSLICE : (i + 1) * VSLICE] = (
            li.reshape(VSLICE, T, B).transpose(1, 2, 0)
        )
    hidden = res.results[0]["h_out"].reshape(H, B).T[None]       # [1, B, H]
    return logits, hidden
